# revision 4
# baseline (speedup 1.0000x reference)
"""AdaptiveConstantEmbeddings distributed Bass kernel for one TRN2 chip.

Reference semantics per domain g (two independent domains):
    e        = max(0, idx - C)                       # [B,S] adaptive row ids
    emb      = adapt_table[e]                        # [B,S,D]
    rel      = emb @ const_table.T                   # [B,S,C]
    out[b,s] = const_table rows where rel == rowmax  # top-1 retrieval

Key algebra: rel rows only depend on e, and e takes at most A distinct
values, so compute R = adapt_table @ const_table.T once per domain
([A,C] instead of [B*S,C], 4x less work), argmax over C per adaptive row
(replaces the dense mask @ const_table matmul, 2x less work), then
out[b,s] = const_table[best[e[b,s]]] is a pure gather.  That is the 8x
algorithmic headroom.

Sharding (8 cores, expert-style): cores 0-3 own domain 0, cores 4-7 own
domain 1.  Within a 4-core group, the A=4096 adaptive rows are split
1024/core for the matmul+argmax, and the B=16 batches are split 4/core
for the output gather.  Each core:
  1. R_shard = adapt_shard @ const.T on TensorE (f32, [1024, 4096])
  2. per 128-row tile: PSUM->SBUF copies on ScalarE, vector.max +
     vector.max_index on VectorE -> best[a] (argmax c per adaptive row)
  3. G_shard[a] = const[best[a]] via indirect DMA gather (overlaps compute)
  4. AllGather G over the 4-core group -> G [4096, 256] (the per-adaptive-row
     answer table)
  5. one dma_gather: rows[t] = G[e[t]] for its 4096 tokens (host-prepped
     wrapped int16 indices), then DMA to the output slice.
"""

import numpy as np

from concourse import bacc, bass, mybir, tile
from concourse.bass_utils import run_bass_kernel_spmd

F32 = mybir.dt.float32
I32 = mybir.dt.int32
I16 = mybir.dt.int16
U16 = mybir.dt.uint16

B, S = 16, 1024
C = 4096          # codebook rows per domain
A = 4096          # adaptive rows per domain
D = 256           # embedding dim
NCORES = 8
GSIZE = 4                     # cores per domain group
GROUPS = [[0, 1, 2, 3], [4, 5, 6, 7]]
ASH = A // GSIZE              # 1024 adaptive rows per core
ATILES = ASH // 128           # 8
KCH = D // 128                # 2 contraction chunks
CTILES = C // 512             # 8 psum column tiles
TOK = (B // GSIZE) * S        # 4096 tokens per core
TJ = TOK // 128               # 32

_NC_CACHE = None


def _build():
    nc = bacc.Bacc("TRN2", target_bir_lowering=False, debug=False, num_devices=NCORES)

    # [adapt_shard.T | const.T] packed so one DMA per k-chunk feeds matmuls
    tabsT = nc.declare_dram_parameter("tabsT", [D, ASH + C], F32, isOutput=False)
    constN = nc.declare_dram_parameter("constN", [C, D], F32, isOutput=False)
    # wrapped dma_gather indices: eidx16[q, s] = e[s*16 + q%16], replicated
    # across the eight 16-partition groups
    eidx16 = nc.declare_dram_parameter("eidx16", [128, TOK // 16], I16, isOutput=False)
    out = nc.declare_dram_parameter("out", [TOK, D], F32, isOutput=True)

    g_loc = nc.dram_tensor("g_loc", [ASH, D], F32)
    g_full = nc.dram_tensor("g_full", [A, D], F32)

    with tile.TileContext(nc) as tc:
        with (
            tc.tile_pool(name="tabs", bufs=1) as tabs_pool,
            tc.tile_pool(name="work", bufs=2) as work,
            tc.tile_pool(name="small", bufs=2) as small,
            tc.tile_pool(name="ps", bufs=8, space="PSUM") as ps,
            tc.tile_pool(name="gather", bufs=1) as gpool,
        ):
            tabs = []
            for k in range(KCH):
                t = tabs_pool.tile([128, ASH + C], F32, name=f"tabs{k}")
                nc.gpsimd.dma_start(t[:], tabsT[k * 128:(k + 1) * 128, :])
                tabs.append(t)

            for T in range(ATILES):
                psums = []
                for c in range(CTILES):
                    p = ps.tile([128, 512], F32, name=f"ps{T}_{c}", tag="ps")
                    psums.append(p)
                for c in range(CTILES):
                    for k in range(KCH):
                        nc.tensor.matmul(
                            psums[c][:],
                            lhsT=tabs[k][:, T * 128:(T + 1) * 128],
                            rhs=tabs[k][:, ASH + c * 512: ASH + (c + 1) * 512],
                            start=(k == 0),
                            stop=(k == KCH - 1),
                        )

                r_sb = work.tile([128, C], F32, name=f"r{T}", tag="r")
                for c in range(CTILES):
                    nc.any.tensor_copy(
                        r_sb[:, c * 512:(c + 1) * 512], psums[c][:],
                    )

                m8 = small.tile([128, 8], F32, name=f"m8_{T}", tag="m8")
                nc.vector.max(out=m8[:], in_=r_sb[:])
                i8 = small.tile([128, 8], U16, name=f"i8_{T}", tag="i8")
                nc.vector.max_index(out=i8[:], in_max=m8[:], in_values=r_sb[:])
                best32 = small.tile([128, 1], I32, name=f"b32_{T}", tag="b32")
                nc.vector.tensor_copy(best32[:], i8[:, :1])

                # G_shard rows for this tile: const[best[a], :]
                g_tile = small.tile([128, D], F32, name=f"g{T}", tag="g")
                nc.gpsimd.indirect_dma_start(
                    out=g_tile[:],
                    out_offset=None,
                    in_=constN[:, :],
                    in_offset=bass.IndirectOffsetOnAxis(ap=best32[:], axis=0),
                )
                nc.sync.dma_start(g_loc[T * 128:(T + 1) * 128, :], g_tile[:])

            nc.gpsimd.collective_compute(
                "AllGather",
                mybir.AluOpType.bypass,
                replica_groups=GROUPS,
                ins=[g_loc[:]],
                outs=[g_full[:]],
            )

            e16 = gpool.tile([128, TOK // 16], I16)
            nc.sync.dma_start(e16[:], eidx16[:])

            # dma_gather crashes the exec unit above ~1024 idxs; chunk it.
            # Chunk k covers tokens [k*1024, (k+1)*1024): its idxs live in
            # e16 columns [k*64, (k+1)*64) (global wrap == local wrap for
            # aligned 1024-token ranges), and rows[p, t, :] is the row for
            # token k*1024 + t*128 + p.
            CH = 1024
            for k in range(TOK // CH):
                rows = gpool.tile([128, CH // 128, D], F32,
                                  name=f"rows{k}", tag="rows", bufs=2)
                nc.gpsimd.dma_gather(
                    out_ap=rows[:],
                    in_ap=g_full[:, :],
                    idxs_ap=e16[:, k * (CH // 16):(k + 1) * (CH // 16)],
                    num_idxs=CH,
                    num_idxs_reg=CH,
                    elem_size=D,
                )
                nc.sync.dma_start(
                    out.ap()[k * CH:(k + 1) * CH].rearrange("(t p) d -> p t d", p=128),
                    rows[:],
                )
    nc.compile()
    return nc


def _get_nc():
    global _NC_CACHE
    if _NC_CACHE is None:
        _NC_CACHE = _build()
    return _NC_CACHE


def _in_maps(idx0, idx1, const_table0, const_table1, adapt_table0, adapt_table1):
    idx = [np.asarray(idx0), np.asarray(idx1)]
    const = [np.ascontiguousarray(np.asarray(const_table0, dtype=np.float32)),
             np.ascontiguousarray(np.asarray(const_table1, dtype=np.float32))]
    adapt = [np.asarray(adapt_table0, dtype=np.float32),
             np.asarray(adapt_table1, dtype=np.float32)]
    constT = [np.ascontiguousarray(c.T) for c in const]

    maps = []
    for core in range(NCORES):
        g, r = divmod(core, GSIZE)
        ash_T = adapt[g][r * ASH:(r + 1) * ASH].T            # [D, ASH]
        tabs = np.concatenate([ash_T, constT[g]], axis=1)    # [D, ASH+C]
        e = idx[g][r * (B // GSIZE):(r + 1) * (B // GSIZE)].reshape(-1)
        e = np.maximum(e.astype(np.int64) - C, 0)            # [TOK]
        ewrap = e.reshape(TOK // 16, 16).T.astype(np.int16)  # [16, TOK//16]
        maps.append({
            "tabsT": np.ascontiguousarray(tabs),
            "constN": const[g],
            "eidx16": np.ascontiguousarray(np.tile(ewrap, (8, 1))),
        })
    return maps


def _run(trace, **inputs):
    nc = _get_nc()
    maps = _in_maps(**inputs)
    res = run_bass_kernel_spmd(nc, maps, core_ids=list(range(NCORES)), trace=trace)
    out = np.empty((2, B, S, D), dtype=np.float32)
    for core in range(NCORES):
        g, r = divmod(core, GSIZE)
        out[g, r * (B // GSIZE):(r + 1) * (B // GSIZE)] = (
            res.results[core]["out"].reshape(B // GSIZE, S, D)
        )
    return out, res


def kernel(**inputs) -> np.ndarray:
    out, _ = _run(False, **inputs)
    return out


def kernel_traced(**inputs):
    """Returns (out, BassKernelResults-with-exec_time_ns) for test harnesses."""
    return _run(True, **inputs)


# revision 14
# speedup vs baseline: 1.1435x; 1.1435x over previous
"""AdaptiveConstantEmbeddings distributed Bass kernel for one TRN2 chip.

Reference semantics per domain g (two independent domains):
    e        = max(0, idx - C)                       # [B,S] adaptive row ids
    emb      = adapt_table[e]                        # [B,S,D]
    rel      = emb @ const_table.T                   # [B,S,C]
    out[b,s] = const_table rows where rel == rowmax  # top-1 retrieval

Key algebra: rel rows only depend on e, and e takes at most A distinct
values, so compute R = adapt_table @ const_table.T once per domain
([A,C] instead of [B*S,C], 4x less work), argmax over C per adaptive row
(replaces the dense mask @ const_table matmul, 2x less work), then
out[b,s] = const_table[best[e[b,s]]] is a pure gather.  That is the 8x
algorithmic headroom.

Sharding (8 cores, expert-style): cores 0-3 own domain 0, cores 4-7 own
domain 1.  Within a 4-core group, the A=4096 adaptive rows are split
1024/core for the matmul+argmax, and the B=16 batches are split 4/core
for the output gather.  Each core:
  1. R_shard = adapt_shard @ const.T on TensorE (f32, [1024, 4096])
  2. per 128-row tile: PSUM->SBUF copies on ScalarE, vector.max +
     vector.max_index on VectorE -> best[a] (argmax c per adaptive row)
  3. G_shard[a] = const[best[a]] via indirect DMA gather (overlaps compute)
  4. AllGather G over the 4-core group -> G [4096, 256] (the per-adaptive-row
     answer table)
  5. one dma_gather: rows[t] = G[e[t]] for its 4096 tokens (host-prepped
     wrapped int16 indices), then DMA to the output slice.
"""

import numpy as np

from concourse import bacc, bass, mybir, tile
from concourse.bass_utils import run_bass_kernel_spmd

F32 = mybir.dt.float32
F32R = mybir.dt.float32r
I32 = mybir.dt.int32
I16 = mybir.dt.int16
U16 = mybir.dt.uint16

B, S = 16, 1024
C = 4096          # codebook rows per domain
A = 4096          # adaptive rows per domain
D = 256           # embedding dim
NCORES = 8
GSIZE = 4                     # cores per domain group
GROUPS = [[0, 1, 2, 3], [4, 5, 6, 7]]
ASH = A // GSIZE              # 1024 adaptive rows per core
ATILES = ASH // 128           # 8
KCH = D // 128                # 2 contraction chunks
CTILES = C // 512             # 8 psum column tiles
TOK = (B // GSIZE) * S        # 4096 tokens per core
TJ = TOK // 128               # 32

_NC_CACHE = None


def _build():
    nc = bacc.Bacc("TRN2", target_bir_lowering=False, debug=False, num_devices=NCORES)

    # [adapt_shard.T | const.T] packed so one DMA per k-chunk feeds matmuls
    tabsT = nc.declare_dram_parameter("tabsT", [D, ASH + C], F32, isOutput=False)
    constN = nc.declare_dram_parameter("constN", [C, D], F32, isOutput=False)
    # wrapped dma_gather indices: eidx16[q, s] = e[s*16 + q%16], replicated
    # across the eight 16-partition groups
    eidx16 = nc.declare_dram_parameter("eidx16", [128, TOK // 16], I16, isOutput=False)
    # out[p, k*8+t, :] = row of token k*1024 + t*128 + p (host unpermutes)
    out = nc.declare_dram_parameter("out", [128, TJ, D], F32, isOutput=True)

    g_loc = nc.dram_tensor("g_loc", [ASH, D], F32)
    g_full = nc.dram_tensor("g_full", [A, D], F32)

    with tile.TileContext(nc) as tc:
        with (
            tc.tile_pool(name="tabs", bufs=1) as tabs_pool,
            tc.tile_pool(name="work", bufs=2) as work,
            tc.tile_pool(name="small", bufs=2) as small,
            tc.tile_pool(name="ps", bufs=8, space="PSUM") as ps,
            tc.tile_pool(name="gather", bufs=1) as gpool,
        ):
            tabs = []
            for k in range(KCH):
                t = tabs_pool.tile([128, ASH + C], F32, name=f"tabs{k}")
                nc.gpsimd.dma_start(t[:], tabsT[k * 128:(k + 1) * 128, :])
                tabs.append(t)

            for T in range(ATILES):
                psums = []
                for c in range(CTILES):
                    p = ps.tile([128, 512], F32, name=f"ps{T}_{c}", tag="ps")
                    psums.append(p)
                for c in range(CTILES):
                    for k in range(KCH):
                        nc.tensor.matmul(
                            psums[c][:],
                            lhsT=tabs[k][:, T * 128:(T + 1) * 128],
                            rhs=tabs[k][:, ASH + c * 512: ASH + (c + 1) * 512],
                            start=(k == 0),
                            stop=(k == KCH - 1),
                        )

                r_sb = work.tile([128, C], F32, name=f"r{T}", tag="r")
                for c in range(CTILES):
                    nc.any.tensor_copy(
                        r_sb[:, c * 512:(c + 1) * 512], psums[c][:],
                    )

                m8 = small.tile([128, 8], F32, name=f"m8_{T}", tag="m8")
                nc.vector.max(out=m8[:], in_=r_sb[:])
                i8 = small.tile([128, 8], U16, name=f"i8_{T}", tag="i8")
                nc.vector.max_index(out=i8[:], in_max=m8[:], in_values=r_sb[:])
                best32 = small.tile([128, 1], I32, name=f"b32_{T}", tag="b32")
                nc.vector.tensor_copy(best32[:], i8[:, :1])

                # G_shard rows for this tile: const[best[a], :]
                g_tile = small.tile([128, D], F32, name=f"g{T}", tag="g")
                nc.gpsimd.indirect_dma_start(
                    out=g_tile[:],
                    out_offset=None,
                    in_=constN[:, :],
                    in_offset=bass.IndirectOffsetOnAxis(ap=best32[:], axis=0),
                )
                nc.sync.dma_start(g_loc[T * 128:(T + 1) * 128, :], g_tile[:])

            # Split AllGather: first half (a-tiles 0-3) overlaps the second
            # half of compute.  Output layout per half h: rows
            # h*2048 + r*512 + (a_local % 512); the host remaps e accordingly.
            half = ASH // 2
            for h in range(2):
                nc.gpsimd.collective_compute(
                    "AllGather",
                    mybir.AluOpType.bypass,
                    replica_groups=GROUPS,
                    ins=[g_loc[h * half:(h + 1) * half]],
                    outs=[g_full[h * 2048:(h + 1) * 2048]],
                )

            e16 = gpool.tile([128, TOK // 16], I16)
            nc.sync.dma_start(e16[:], eidx16[:])

            # dma_gather crashes the exec unit above ~1024 idxs; chunk it.
            # Chunk k covers tokens [k*1024, (k+1)*1024): its idxs live in
            # e16 columns [k*64, (k+1)*64) (global wrap == local wrap for
            # aligned 1024-token ranges), and rows[p, t, :] is the row for
            # token k*1024 + t*128 + p.
            CH = 1024
            out_eng = [nc.sync, nc.scalar, nc.sync, nc.scalar]
            for k in range(TOK // CH):
                rows = gpool.tile([128, CH // 128, D], F32,
                                  name=f"rows{k}", tag=f"rows{k}", bufs=1)
                nc.gpsimd.dma_gather(
                    out_ap=rows[:],
                    in_ap=g_full[:, :],
                    idxs_ap=e16[:, k * (CH // 16):(k + 1) * (CH // 16)],
                    num_idxs=CH,
                    num_idxs_reg=CH,
                    elem_size=D,
                )
                out_eng[k % 4].dma_start(
                    out[:, k * (CH // 128):(k + 1) * (CH // 128), :], rows[:]
                )
    nc.compile()
    return nc


def _get_nc():
    global _NC_CACHE
    if _NC_CACHE is None:
        _NC_CACHE = _build()
    return _NC_CACHE


def _in_maps(idx0, idx1, const_table0, const_table1, adapt_table0, adapt_table1):
    idx = [np.asarray(idx0), np.asarray(idx1)]
    const = [np.ascontiguousarray(np.asarray(const_table0, dtype=np.float32)),
             np.ascontiguousarray(np.asarray(const_table1, dtype=np.float32))]
    adapt = [np.asarray(adapt_table0, dtype=np.float32),
             np.asarray(adapt_table1, dtype=np.float32)]
    constT = [np.ascontiguousarray(c.T) for c in const]

    maps = []
    for core in range(NCORES):
        g, r = divmod(core, GSIZE)
        ash_T = adapt[g][r * ASH:(r + 1) * ASH].T            # [D, ASH]
        tabs = np.concatenate([ash_T, constT[g]], axis=1)    # [D, ASH+C]
        e = idx[g][r * (B // GSIZE):(r + 1) * (B // GSIZE)].reshape(-1)
        e = np.maximum(e.astype(np.int64) - C, 0)            # [TOK] global a-ids
        # remap into the split-AllGather g_full layout:
        # a = r*1024 + al  ->  (al//512)*2048 + r*512 + (al % 512)
        ra, al = np.divmod(e, ASH)
        e = (al // 512) * 2048 + ra * 512 + (al % 512)
        ewrap = e.reshape(TOK // 16, 16).T.astype(np.int16)  # [16, TOK//16]
        maps.append({
            "tabsT": np.ascontiguousarray(tabs),
            "constN": const[g],
            "eidx16": np.ascontiguousarray(np.tile(ewrap, (8, 1))),
        })
    return maps


def _token_of_pos():
    # device writes out[p, k*8+t, :] = token k*1024 + t*128 + p
    p = np.arange(128)[:, None]
    kt = np.arange(TJ)[None, :]
    return ((kt // 8) * 1024 + (kt % 8) * 128 + p).reshape(-1)


_TOKEN_OF_POS = _token_of_pos()


def _run(trace, **inputs):
    nc = _get_nc()
    maps = _in_maps(**inputs)
    res = run_bass_kernel_spmd(nc, maps, core_ids=list(range(NCORES)), trace=trace)
    out = np.empty((2, B, S, D), dtype=np.float32)
    for core in range(NCORES):
        g, r = divmod(core, GSIZE)
        rows = np.empty((TOK, D), dtype=np.float32)
        rows[_TOKEN_OF_POS] = res.results[core]["out"].reshape(TOK, D)
        out[g, r * (B // GSIZE):(r + 1) * (B // GSIZE)] = rows.reshape(
            B // GSIZE, S, D
        )
    return out, res


def kernel(**inputs) -> np.ndarray:
    out, _ = _run(False, **inputs)
    return out


def kernel_traced(**inputs):
    """Returns (out, BassKernelResults-with-exec_time_ns) for test harnesses."""
    return _run(True, **inputs)


# revision 18
# speedup vs baseline: 1.1947x; 1.0447x over previous
"""AdaptiveConstantEmbeddings distributed Bass kernel for one TRN2 chip.

Reference semantics per domain g (two independent domains):
    e        = max(0, idx - C)                       # [B,S] adaptive row ids
    emb      = adapt_table[e]                        # [B,S,D]
    rel      = emb @ const_table.T                   # [B,S,C]
    out[b,s] = const_table rows where rel == rowmax  # top-1 retrieval

Key algebra: rel rows only depend on e, and e takes at most A distinct
values, so compute R = adapt_table @ const_table.T once per domain
([A,C] instead of [B*S,C], 4x less work), argmax over C per adaptive row
(replaces the dense mask @ const_table matmul, 2x less work), then
out[b,s] = const_table[best[e[b,s]]] is a pure gather.  That is the 8x
algorithmic headroom.

Sharding (8 cores, expert-style): cores 0-3 own domain 0, cores 4-7 own
domain 1.  Within a 4-core group, the A=4096 adaptive rows are split
1024/core for the matmul+argmax, and the B=16 batches are split 4/core
for the output gather.  Each core:
  1. R_shard = adapt_shard @ const.T on TensorE (f32, [1024, 4096])
  2. per 128-row tile: PSUM->SBUF copies on ScalarE, vector.max +
     vector.max_index on VectorE -> best[a] (argmax c per adaptive row)
  3. G_shard[a] = const[best[a]] via indirect DMA gather (overlaps compute)
  4. AllGather G over the 4-core group -> G [4096, 256] (the per-adaptive-row
     answer table)
  5. one dma_gather: rows[t] = G[e[t]] for its 4096 tokens (host-prepped
     wrapped int16 indices), then DMA to the output slice.
"""

import numpy as np

from concourse import bacc, bass, mybir, tile
from concourse.bass_utils import run_bass_kernel_spmd

F32 = mybir.dt.float32
F32R = mybir.dt.float32r
I32 = mybir.dt.int32
I16 = mybir.dt.int16
U16 = mybir.dt.uint16

B, S = 16, 1024
C = 4096          # codebook rows per domain
A = 4096          # adaptive rows per domain
D = 256           # embedding dim
NCORES = 8
GSIZE = 4                     # cores per domain group
GROUPS = [[0, 1, 2, 3], [4, 5, 6, 7]]
ASH = A // GSIZE              # 1024 adaptive rows per core
ATILES = ASH // 128           # 8
KCH = D // 128                # 2 contraction chunks
CTILES = C // 512             # 8 psum column tiles
TOK = (B // GSIZE) * S        # 4096 tokens per core
TJ = TOK // 128               # 32

_NC_CACHE = None


def _build():
    nc = bacc.Bacc("TRN2", target_bir_lowering=False, debug=False, num_devices=NCORES)

    # [adapt_shard.T | const.T] packed so one DMA per k-chunk feeds matmuls
    tabsT = nc.declare_dram_parameter("tabsT", [D, ASH + C], F32, isOutput=False)
    constN = nc.declare_dram_parameter("constN", [C, D], F32, isOutput=False)
    # wrapped dma_gather indices: eidx16[q, s] = e[s*16 + q%16], replicated
    # across the eight 16-partition groups
    eidx16 = nc.declare_dram_parameter("eidx16", [128, TOK // 16], I16, isOutput=False)
    # out[p, k*8+t, :] = row of token k*1024 + t*128 + p (host unpermutes)
    out = nc.declare_dram_parameter("out", [128, TJ, D], F32, isOutput=True)

    g_loc = nc.dram_tensor("g_loc", [ASH, D], F32)
    g_full = nc.dram_tensor("g_full", [A, D], F32)

    with tile.TileContext(nc) as tc:
        with (
            tc.tile_pool(name="tabs", bufs=1) as tabs_pool,
            tc.tile_pool(name="work", bufs=2) as work,
            tc.tile_pool(name="small", bufs=2) as small,
            tc.tile_pool(name="ps", bufs=8, space="PSUM") as ps,
            tc.tile_pool(name="gather", bufs=1) as gpool,
        ):
            tabs = []
            for k in range(KCH):
                t = tabs_pool.tile([128, ASH + C], F32, name=f"tabs{k}")
                nc.gpsimd.dma_start(t[:], tabsT[k * 128:(k + 1) * 128, :])
                tabs.append(t)

            for T in range(ATILES):
                psums = []
                for c in range(CTILES):
                    p = ps.tile([128, 512], F32, name=f"ps{T}_{c}", tag="ps")
                    psums.append(p)
                for c in range(CTILES):
                    for k in range(KCH):
                        nc.tensor.matmul(
                            psums[c][:],
                            lhsT=tabs[k][:, T * 128:(T + 1) * 128],
                            rhs=tabs[k][:, ASH + c * 512: ASH + (c + 1) * 512],
                            start=(k == 0),
                            stop=(k == KCH - 1),
                        )

                r_sb = work.tile([128, C], F32, name=f"r{T}", tag="r")
                for c in range(CTILES):
                    nc.any.tensor_copy(
                        r_sb[:, c * 512:(c + 1) * 512], psums[c][:],
                    )

                m8 = small.tile([128, 8], F32, name=f"m8_{T}", tag="m8")
                nc.vector.max(out=m8[:], in_=r_sb[:])
                i8 = small.tile([128, 8], U16, name=f"i8_{T}", tag="i8")
                nc.vector.max_index(out=i8[:], in_max=m8[:], in_values=r_sb[:])
                best32 = small.tile([128, 1], I32, name=f"b32_{T}", tag="b32")
                nc.vector.tensor_copy(best32[:], i8[:, :1])

                # G_shard rows for this tile: const[best[a], :]
                g_tile = small.tile([128, D], F32, name=f"g{T}", tag="g")
                nc.gpsimd.indirect_dma_start(
                    out=g_tile[:],
                    out_offset=None,
                    in_=constN[:, :],
                    in_offset=bass.IndirectOffsetOnAxis(ap=best32[:], axis=0),
                )
                nc.sync.dma_start(g_loc[T * 128:(T + 1) * 128, :], g_tile[:])

            # Split AllGather: first half (a-tiles 0-3) overlaps the second
            # half of compute.  Output layout per half h: rows
            # h*2048 + r*512 + (a_local % 512); the host remaps e accordingly.
            half = ASH // 2
            for h in range(2):
                nc.gpsimd.collective_compute(
                    "AllGather",
                    mybir.AluOpType.bypass,
                    replica_groups=GROUPS,
                    ins=[g_loc[h * half:(h + 1) * half]],
                    outs=[g_full[h * 2048:(h + 1) * 2048]],
                )

            e16 = gpool.tile([128, TOK // 16], I16)
            nc.sync.dma_start(e16[:], eidx16[:])

            # dma_gather crashes the exec unit above ~1024 idxs; chunk it.
            # Chunk k covers tokens [k*1024, (k+1)*1024): its idxs live in
            # e16 columns [k*64, (k+1)*64) (global wrap == local wrap for
            # aligned 1024-token ranges), and rows[p, t, :] is the row for
            # token k*1024 + t*128 + p.
            CH = 1024
            out_eng = [nc.sync, nc.scalar, nc.sync, nc.scalar]
            for k in range(TOK // CH):
                rows = gpool.tile([128, CH // 128, D], F32,
                                  name=f"rows{k}", tag=f"rows{k}", bufs=1)
                nc.gpsimd.dma_gather(
                    out_ap=rows[:],
                    in_ap=g_full[:, :],
                    idxs_ap=e16[:, k * (CH // 16):(k + 1) * (CH // 16)],
                    num_idxs=CH,
                    num_idxs_reg=CH,
                    elem_size=D,
                )
                out_eng[k % 4].dma_start(
                    out[:, k * (CH // 128):(k + 1) * (CH // 128), :], rows[:]
                )
    nc.compile()
    return nc


def _get_nc():
    global _NC_CACHE
    if _NC_CACHE is None:
        _NC_CACHE = _build()
    return _NC_CACHE


def _in_maps(idx0, idx1, const_table0, const_table1, adapt_table0, adapt_table1):
    idx = [np.asarray(idx0), np.asarray(idx1)]
    const = [np.ascontiguousarray(np.asarray(const_table0, dtype=np.float32)),
             np.ascontiguousarray(np.asarray(const_table1, dtype=np.float32))]
    adapt = [np.asarray(adapt_table0, dtype=np.float32),
             np.asarray(adapt_table1, dtype=np.float32)]
    constT = [np.ascontiguousarray(c.T) for c in const]

    maps = []
    orders = []
    for core in range(NCORES):
        g, r = divmod(core, GSIZE)
        ash_T = adapt[g][r * ASH:(r + 1) * ASH].T            # [D, ASH]
        tabs = np.concatenate([ash_T, constT[g]], axis=1)    # [D, ASH+C]
        e = idx[g][r * (B // GSIZE):(r + 1) * (B // GSIZE)].reshape(-1)
        e = np.maximum(e.astype(np.int64) - C, 0)            # [TOK] global a-ids
        # remap into the split-AllGather g_full layout:
        # a = r*1024 + al  ->  (al//512)*2048 + r*512 + (al % 512)
        ra, al = np.divmod(e, ASH)
        e = (al // 512) * 2048 + ra * 512 + (al % 512)
        # sort tokens by table row so gather descriptors walk HBM rows in
        # ascending order (duplicates hit the row buffer); host unpermutes
        order = np.argsort(e, kind="stable").astype(np.int64)
        ewrap = e[order].reshape(TOK // 16, 16).T.astype(np.int16)
        maps.append({
            "tabsT": np.ascontiguousarray(tabs),
            "constN": const[g],
            "eidx16": np.ascontiguousarray(np.tile(ewrap, (8, 1))),
        })
        orders.append(order)
    return maps, orders


def _token_of_pos():
    # device writes out[p, k*8+t, :] = token k*1024 + t*128 + p
    p = np.arange(128)[:, None]
    kt = np.arange(TJ)[None, :]
    return ((kt // 8) * 1024 + (kt % 8) * 128 + p).reshape(-1)


_TOKEN_OF_POS = _token_of_pos()


def _run(trace, **inputs):
    nc = _get_nc()
    maps, orders = _in_maps(**inputs)
    res = run_bass_kernel_spmd(nc, maps, core_ids=list(range(NCORES)), trace=trace)
    out = np.empty((2, B, S, D), dtype=np.float32)
    for core in range(NCORES):
        g, r = divmod(core, GSIZE)
        rows = np.empty((TOK, D), dtype=np.float32)
        rows[orders[core][_TOKEN_OF_POS]] = res.results[core]["out"].reshape(TOK, D)
        out[g, r * (B // GSIZE):(r + 1) * (B // GSIZE)] = rows.reshape(
            B // GSIZE, S, D
        )
    return out, res


def kernel(**inputs) -> np.ndarray:
    out, _ = _run(False, **inputs)
    return out


def kernel_traced(**inputs):
    """Returns (out, BassKernelResults-with-exec_time_ns) for test harnesses."""
    return _run(True, **inputs)


# revision 19
# speedup vs baseline: 1.2229x; 1.0236x over previous
"""AdaptiveConstantEmbeddings distributed Bass kernel for one TRN2 chip.

Reference semantics per domain g (two independent domains):
    e        = max(0, idx - C)                       # [B,S] adaptive row ids
    emb      = adapt_table[e]                        # [B,S,D]
    rel      = emb @ const_table.T                   # [B,S,C]
    out[b,s] = const_table rows where rel == rowmax  # top-1 retrieval

Key algebra: rel rows only depend on e, and e takes at most A distinct
values, so compute R = adapt_table @ const_table.T once per domain
([A,C] instead of [B*S,C], 4x less work), argmax over C per adaptive row
(replaces the dense mask @ const_table matmul, 2x less work), then
out[b,s] = const_table[best[e[b,s]]] is a pure gather.  That is the 8x
algorithmic headroom.

Sharding (8 cores, expert-style): cores 0-3 own domain 0, cores 4-7 own
domain 1.  Within a 4-core group, the A=4096 adaptive rows are split
1024/core for the matmul+argmax, and the B=16 batches are split 4/core
for the output gather.  Each core:
  1. R_shard = adapt_shard @ const.T on TensorE (f32, [1024, 4096])
  2. per 128-row tile: PSUM->SBUF copies on ScalarE, vector.max +
     vector.max_index on VectorE -> best[a] (argmax c per adaptive row)
  3. G_shard[a] = const[best[a]] via indirect DMA gather (overlaps compute)
  4. AllGather G over the 4-core group -> G [4096, 256] (the per-adaptive-row
     answer table)
  5. one dma_gather: rows[t] = G[e[t]] for its 4096 tokens (host-prepped
     wrapped int16 indices), then DMA to the output slice.
"""

import numpy as np

from concourse import bacc, bass, mybir, tile
from concourse.bass_utils import run_bass_kernel_spmd

F32 = mybir.dt.float32
F32R = mybir.dt.float32r
I32 = mybir.dt.int32
I16 = mybir.dt.int16
U16 = mybir.dt.uint16

B, S = 16, 1024
C = 4096          # codebook rows per domain
A = 4096          # adaptive rows per domain
D = 256           # embedding dim
NCORES = 8
GSIZE = 4                     # cores per domain group
GROUPS = [[0, 1, 2, 3], [4, 5, 6, 7]]
ASH = A // GSIZE              # 1024 adaptive rows per core
ATILES = ASH // 128           # 8
KCH = D // 128                # 2 contraction chunks
CTILES = C // 512             # 8 psum column tiles
TOK = (B // GSIZE) * S        # 4096 tokens per core
TJ = TOK // 128               # 32

_NC_CACHE = None


def _build():
    nc = bacc.Bacc("TRN2", target_bir_lowering=False, debug=False, num_devices=NCORES)

    # [adapt_shard.T | const.T] packed so one DMA per k-chunk feeds matmuls
    tabsT = nc.declare_dram_parameter("tabsT", [D, ASH + C], F32, isOutput=False)
    constN = nc.declare_dram_parameter("constN", [C, D], F32, isOutput=False)
    # wrapped dma_gather indices: eidx16[q, s] = e[s*16 + q%16], replicated
    # across the eight 16-partition groups
    eidx16 = nc.declare_dram_parameter("eidx16", [128, TOK // 16], I16, isOutput=False)
    # out[p, k*8+t, :] = row of token k*1024 + t*128 + p (host unpermutes)
    out = nc.declare_dram_parameter("out", [128, TJ, D], F32, isOutput=True)

    g_loc = nc.dram_tensor("g_loc", [ASH, D], F32)
    g_full = nc.dram_tensor("g_full", [A, D], F32)

    with tile.TileContext(nc) as tc:
        with (
            tc.tile_pool(name="tabs", bufs=1) as tabs_pool,
            tc.tile_pool(name="work", bufs=2) as work,
            tc.tile_pool(name="small", bufs=2) as small,
            tc.tile_pool(name="ps", bufs=8, space="PSUM") as ps,
            tc.tile_pool(name="gather", bufs=1) as gpool,
        ):
            # split loads so the first matmuls start after ~1.5MB, not 5MB
            tabs = []
            for k in range(KCH):
                t = tabs_pool.tile([128, ASH + C], F32, name=f"tabs{k}")
                nc.gpsimd.dma_start(t[:, :ASH], tabsT[k * 128:(k + 1) * 128, :ASH])
                for c in range(CTILES):
                    nc.gpsimd.dma_start(
                        t[:, ASH + c * 512: ASH + (c + 1) * 512],
                        tabsT[k * 128:(k + 1) * 128, ASH + c * 512: ASH + (c + 1) * 512],
                    )
                tabs.append(t)

            for T in range(ATILES):
                psums = []
                for c in range(CTILES):
                    p = ps.tile([128, 512], F32, name=f"ps{T}_{c}", tag="ps")
                    psums.append(p)
                for c in range(CTILES):
                    for k in range(KCH):
                        nc.tensor.matmul(
                            psums[c][:],
                            lhsT=tabs[k][:, T * 128:(T + 1) * 128],
                            rhs=tabs[k][:, ASH + c * 512: ASH + (c + 1) * 512],
                            start=(k == 0),
                            stop=(k == KCH - 1),
                        )

                r_sb = work.tile([128, C], F32, name=f"r{T}", tag="r")
                for c in range(CTILES):
                    nc.any.tensor_copy(
                        r_sb[:, c * 512:(c + 1) * 512], psums[c][:],
                    )

                m8 = small.tile([128, 8], F32, name=f"m8_{T}", tag="m8")
                nc.vector.max(out=m8[:], in_=r_sb[:])
                i8 = small.tile([128, 8], U16, name=f"i8_{T}", tag="i8")
                nc.vector.max_index(out=i8[:], in_max=m8[:], in_values=r_sb[:])
                best32 = small.tile([128, 1], I32, name=f"b32_{T}", tag="b32")
                nc.vector.tensor_copy(best32[:], i8[:, :1])

                # G_shard rows for this tile: const[best[a], :]
                g_tile = small.tile([128, D], F32, name=f"g{T}", tag="g")
                nc.gpsimd.indirect_dma_start(
                    out=g_tile[:],
                    out_offset=None,
                    in_=constN[:, :],
                    in_offset=bass.IndirectOffsetOnAxis(ap=best32[:], axis=0),
                )
                nc.sync.dma_start(g_loc[T * 128:(T + 1) * 128, :], g_tile[:])

            # Split AllGather: first half (a-tiles 0-3) overlaps the second
            # half of compute.  Output layout per half h: rows
            # h*2048 + r*512 + (a_local % 512); the host remaps e accordingly.
            half = ASH // 2
            for h in range(2):
                nc.gpsimd.collective_compute(
                    "AllGather",
                    mybir.AluOpType.bypass,
                    replica_groups=GROUPS,
                    ins=[g_loc[h * half:(h + 1) * half]],
                    outs=[g_full[h * 2048:(h + 1) * 2048]],
                )

            e16 = gpool.tile([128, TOK // 16], I16)
            nc.sync.dma_start(e16[:], eidx16[:])

            # dma_gather crashes the exec unit above ~1024 idxs; chunk it.
            # Chunk k covers tokens [k*1024, (k+1)*1024): its idxs live in
            # e16 columns [k*64, (k+1)*64) (global wrap == local wrap for
            # aligned 1024-token ranges), and rows[p, t, :] is the row for
            # token k*1024 + t*128 + p.
            CH = 1024
            out_eng = [nc.sync, nc.scalar, nc.sync, nc.scalar]
            for k in range(TOK // CH):
                rows = gpool.tile([128, CH // 128, D], F32,
                                  name=f"rows{k}", tag=f"rows{k}", bufs=1)
                nc.gpsimd.dma_gather(
                    out_ap=rows[:],
                    in_ap=g_full[:, :],
                    idxs_ap=e16[:, k * (CH // 16):(k + 1) * (CH // 16)],
                    num_idxs=CH,
                    num_idxs_reg=CH,
                    elem_size=D,
                )
                out_eng[k % 4].dma_start(
                    out[:, k * (CH // 128):(k + 1) * (CH // 128), :], rows[:]
                )
    nc.compile()
    return nc


def _get_nc():
    global _NC_CACHE
    if _NC_CACHE is None:
        _NC_CACHE = _build()
    return _NC_CACHE


def _in_maps(idx0, idx1, const_table0, const_table1, adapt_table0, adapt_table1):
    idx = [np.asarray(idx0), np.asarray(idx1)]
    const = [np.ascontiguousarray(np.asarray(const_table0, dtype=np.float32)),
             np.ascontiguousarray(np.asarray(const_table1, dtype=np.float32))]
    adapt = [np.asarray(adapt_table0, dtype=np.float32),
             np.asarray(adapt_table1, dtype=np.float32)]
    constT = [np.ascontiguousarray(c.T) for c in const]

    maps = []
    orders = []
    for core in range(NCORES):
        g, r = divmod(core, GSIZE)
        ash_T = adapt[g][r * ASH:(r + 1) * ASH].T            # [D, ASH]
        tabs = np.concatenate([ash_T, constT[g]], axis=1)    # [D, ASH+C]
        e = idx[g][r * (B // GSIZE):(r + 1) * (B // GSIZE)].reshape(-1)
        e = np.maximum(e.astype(np.int64) - C, 0)            # [TOK] global a-ids
        # remap into the split-AllGather g_full layout:
        # a = r*1024 + al  ->  (al//512)*2048 + r*512 + (al % 512)
        ra, al = np.divmod(e, ASH)
        e = (al // 512) * 2048 + ra * 512 + (al % 512)
        # sort tokens by table row so gather descriptors walk HBM rows in
        # ascending order (duplicates hit the row buffer); host unpermutes
        order = np.argsort(e, kind="stable").astype(np.int64)
        ewrap = e[order].reshape(TOK // 16, 16).T.astype(np.int16)
        maps.append({
            "tabsT": np.ascontiguousarray(tabs),
            "constN": const[g],
            "eidx16": np.ascontiguousarray(np.tile(ewrap, (8, 1))),
        })
        orders.append(order)
    return maps, orders


def _token_of_pos():
    # device writes out[p, k*8+t, :] = token k*1024 + t*128 + p
    p = np.arange(128)[:, None]
    kt = np.arange(TJ)[None, :]
    return ((kt // 8) * 1024 + (kt % 8) * 128 + p).reshape(-1)


_TOKEN_OF_POS = _token_of_pos()


def _run(trace, **inputs):
    nc = _get_nc()
    maps, orders = _in_maps(**inputs)
    res = run_bass_kernel_spmd(nc, maps, core_ids=list(range(NCORES)), trace=trace)
    out = np.empty((2, B, S, D), dtype=np.float32)
    for core in range(NCORES):
        g, r = divmod(core, GSIZE)
        rows = np.empty((TOK, D), dtype=np.float32)
        rows[orders[core][_TOKEN_OF_POS]] = res.results[core]["out"].reshape(TOK, D)
        out[g, r * (B // GSIZE):(r + 1) * (B // GSIZE)] = rows.reshape(
            B // GSIZE, S, D
        )
    return out, res


def kernel(**inputs) -> np.ndarray:
    out, _ = _run(False, **inputs)
    return out


def kernel_traced(**inputs):
    """Returns (out, BassKernelResults-with-exec_time_ns) for test harnesses."""
    return _run(True, **inputs)
